# revision 37
# baseline (speedup 1.0000x reference)
"""CAMIL self-attention Trainium2 kernel (8 NeuronCores, SPMD).

Reference computation (B=2, N=8192, IN_DIM=ATT_DIM=512):
    q = X @ Wq ; k = X @ Wk ; v = X @ Wv
    w_i = inv_scale * m_i * sum_d q[i,d] * (adj @ (k*m))[i,d]
    L   = softmax(w, axis=bag)[:, :, None] * v

Sharding: 8 cores = (batch 2) x (4 row-blocks of 2048). Each core holds
adj[b, i_block, :]^T (pre-transposed on host so the contraction dim j lands
on SBUF partitions), computes its w-slice on-device, the 4 cores of each
batch AllGather w (8 KB/core, one collective), every core computes the
softmax normalizers locally, then scales its own v rows.

Precision strategy (validated vs the reference in numpy: rel err 8.3e-3 vs
the 2e-2 gate; HW matches the numpy sim exactly since all inputs are
quantized on host): adj, X, Wq, Wk and k_m are all fp8e4m3 so every w-path
matmul runs in DoubleRow mode (K=256/instruction, 2x bf16 throughput); the
softmax over the bag is near one-hot (top-2 gaps ~11-16) so w tolerates
absolute errors of ~3 with small output impact. v = X @ Wv runs in bf16
and L is stored bf16 (host upcasts) since the gate is 2e-2 relative.

Schedule notes (from perfetto traces of prior iterations):
- Matmuls are issued chunk-major so consecutive MMs hit different PSUM
  banks and overlap in the PE pipeline (same-bank accumulation serializes
  at ~477ns/MM vs ~264ns overlapped).
- PSUM drains are split across the vector and scalar engines so neither
  gates PSUM-bank recycling (a lone vector drain at ~654ns/tile makes
  phase 1 vector-bound).
- The v projection runs AFTER the big matmul so its PE work covers the
  single w-AllGather's 15-25us collective latency (one CC op, warmed at
  startup: CC ops are latency-dominated and variable, so fewer is better).
- Softmax partition reductions use PE-transpose + ones-matmuls instead of
  gpsimd partition_all_reduce (avoids a 7us gpsimd library load), and keep
  the scalar engine on EXP only (an EXP->LN switch costs a 1.5us ACT-table
  load).
- The first adj tiles are prefetched before the q projection; bulk const
  loads ride the gpsimd DMA queue; the v-input load rides the sync queue
  behind the adj stream — the xt8/adj streams never stall.
"""

import numpy as np
from contextlib import ExitStack

import concourse.bass as bass
import concourse.bacc as bacc
import concourse.tile as tile
from concourse import mybir, bass_isa
from concourse.bass_utils import run_bass_kernel_spmd

F32 = mybir.dt.float32
BF16 = mybir.dt.bfloat16
FP8 = mybir.dt.float8e4

B, N, D = 2, 8192, 512
RPC = N // 4  # rows per core: 2048
INV_SCALE = float(1.0 / np.sqrt(np.float32(D)))
GROUPS = [[0, 1, 2, 3], [4, 5, 6, 7]]
DR = mybir.MatmulPerfMode.DoubleRow

_CACHE = {}


def _build(stage="full"):
    # stage: debug gate — "p1" (projections only), "p2" (+w), "coll" (+gather),
    # "full" (everything). kernel() always uses "full".
    nc = bacc.Bacc(None, target_bir_lowering=False, debug=False, num_devices=8)

    adjt = nc.dram_tensor("adjt", [N, RPC], FP8, kind="ExternalInput")
    xt8 = nc.dram_tensor("xt8", [D, N], FP8, kind="ExternalInput")
    xq8 = nc.dram_tensor("xq8", [D, RPC], FP8, kind="ExternalInput")
    xvt = nc.dram_tensor("xvt", [D, RPC], BF16, kind="ExternalInput")
    wq8d = nc.dram_tensor("wq8", [D, D], FP8, kind="ExternalInput")
    wk8d = nc.dram_tensor("wk8", [D, D], FP8, kind="ExternalInput")
    wv16d = nc.dram_tensor("wv16", [D, D], BF16, kind="ExternalInput")
    maskqd = nc.dram_tensor("maskq", [128, 64], F32, kind="ExternalInput")
    maskownd = nc.dram_tensor("maskown", [128, 16], F32, kind="ExternalInput")
    identd = nc.dram_tensor("ident", [128, 128], F32, kind="ExternalInput")
    loutd = nc.dram_tensor("lout", [RPC, D], BF16, kind="ExternalOutput")

    w_locd = nc.dram_tensor("w_loc", [128, 16], F32)
    w_alld = nc.dram_tensor("w_all", [4, 128, 16], F32)
    dwarm_in = nc.dram_tensor("dwarm_in", [1, 16], F32)
    dwarm_out = nc.dram_tensor("dwarm_out", [4, 1, 16], F32)

    MUL = mybir.AluOpType.mult
    X_AX = mybir.AxisListType.X
    EXP = mybir.ActivationFunctionType.Exp

    with tile.TileContext(nc) as tc, ExitStack() as ctx:
        wtail = ctx.enter_context(tc.tile_pool(name="wtail", bufs=1))
        bigctx = ExitStack()
        big = bigctx.enter_context(tc.tile_pool(name="big", bufs=1))

        km_s = big.tile([128, 64, D], FP8)       # k*mask, [j-part, j-chunk, d]
        q_s = big.tile([128, 16, D], F32)        # own q rows
        wq_s = big.tile([128, 4, D], FP8)
        wk_s = big.tile([128, 4, D], FP8)
        maskq_s = big.tile([128, 64], F32)

        w_sb = wtail.tile([128, 16], F32)        # own w (pre-mask)
        w2 = wtail.tile([128, 16], F32)          # own w (masked)
        maskown_s = wtail.tile([128, 16], F32)
        ident_s = wtail.tile([128, 128], F32)
        ones_c = wtail.tile([128, 1], F32)       # column of ones (dot lhsT)
        ones_r = wtail.tile([1, 128], F32)       # row of ones (broadcast lhsT)
        negones_r = wtail.tile([1, 128], F32)    # row of -1 (negating broadcast)
        wv_s = wtail.tile([128, 4, D], BF16)
        xv_s = wtail.tile([128, 4, RPC], BF16)   # own X^T bf16 for v
        v_sb = wtail.tile([128, 16, D], BF16)    # own v rows

        def _late_consts():
            # deferred constant loads on the gpsimd DMA queue so they never
            # stall the xt8 stream; plus exp-LUT warmup on idle engines
            nc.gpsimd.dma_start(wq_s[:], wq8d[:].rearrange("(c p) d -> p c d", p=128))
            nc.gpsimd.dma_start(wv_s[:], wv16d[:].rearrange("(c p) d -> p c d", p=128))
            nc.gpsimd.dma_start(maskown_s[:], maskownd[:])
            nc.gpsimd.dma_start(ident_s[:], identd[:])
            nc.vector.memset(ones_c[:], 1.0)
            nc.vector.memset(ones_r[:], 1.0)
            nc.vector.memset(negones_r[:], -1.0)
            warm = wtail.tile([128, 16], F32, name="warm")
            nc.vector.memset(warm[:], 0.0)
            nc.scalar.activation(out=warm[:], in_=warm[:], func=EXP,
                                 bias=0.0, scale=1.0)
            # collective firmware warmup so the real w-gather is warm
            nc.gpsimd.dma_start(dwarm_in[:], warm[0:1, :])
            nc.gpsimd.collective_compute(
                "AllGather",
                mybir.AluOpType.bypass,
                replica_groups=GROUPS,
                ins=[dwarm_in[:]],
                outs=[dwarm_out[:]],
            )

        # ---- Phase 1: k_m (all N rows) and q (own rows), fp8 DoubleRow ----
        s2ctx = ExitStack()
        s2pool = s2ctx.enter_context(tc.tile_pool(name="s2", bufs=24))
        with (
            tc.tile_pool(name="p1", bufs=8) as p1pool,
            tc.tile_pool(name="ps1", bufs=8, space="PSUM") as ps1,
        ):
            # first consts ride the idle scalar/gpsimd queues so the sync
            # queue's first (cold, ~1.5us) DMAs are the xt8 panel halves
            nc.scalar.dma_start(wk_s[:], wk8d[:].rearrange("(c p) d -> p c d", p=128))
            nc.gpsimd.dma_start(maskq_s[:], maskqd[:])
            # xq8 up front on scalar, before the k-loop's scalar ACT drains
            # queue behind them (they'd otherwise delay the q projection)
            xq_tiles = []
            for gp in range(4):
                xq_t = p1pool.tile([128, 4, 512], FP8, tag="xqp", name=f"xq_{gp}")
                nc.scalar.dma_start(
                    xq_t[:],
                    xq8[:, gp * 512:(gp + 1) * 512].rearrange("(c p) j -> p c j", p=128),
                )
                xq_tiles.append(xq_t)
            for jp in range(16):  # panels of 512 bag rows (split DMA halves
                # so the first DR matmul starts after 128KB, not 256KB)
                xt_t = p1pool.tile([128, 4, 512], FP8, tag="xtp")
                src = xt8[:, jp * 512:(jp + 1) * 512].rearrange("(c p) j -> p c j", p=128)
                nc.sync.dma_start(xt_t[:, 0:2, :], src[:, 0:2, :])
                nc.sync.dma_start(xt_t[:, 2:4, :], src[:, 2:4, :])
                if jp == 1:
                    _late_consts()
                # chunk-major matmul order so consecutive MMs hit different
                # PSUM banks and overlap in the PE pipeline
                psks = [ps1.tile([128, D], F32, tag="psk", name=f"psk_{jp}_{i}")
                        for i in range(4)]
                for u in range(2):
                    for jc2 in range(4):
                        nc.tensor.matmul(
                            psks[jc2][:],
                            lhsT=xt_t[:, 2 * u:2 * u + 2, jc2 * 128:(jc2 + 1) * 128],
                            rhs=wk_s[:, 2 * u:2 * u + 2, :],
                            start=(u == 0),
                            stop=(u == 1),
                            perf_mode=DR,
                        )
                for jc2 in range(4):
                    jc = jp * 4 + jc2
                    # PSUM drains split across vector + scalar so neither
                    # engine gates the PE's PSUM-bank recycling
                    if jc2 % 2 == 0:
                        nc.vector.tensor_scalar_mul(km_s[:, jc, :], psks[jc2][:], maskq_s[:, jc:jc + 1])
                    else:
                        nc.scalar.activation(out=km_s[:, jc, :], in_=psks[jc2][:],
                                             func=mybir.ActivationFunctionType.Copy,
                                             scale=maskq_s[:, jc:jc + 1])
            # prefetch the first adj tiles so phase 2 starts right after q
            pre_adj = []
            for jb in range(2):
                at = s2pool.tile([128, 4, 512], FP8, tag="adjs", name=f"adjpre{jb}")
                nc.sync.dma_start(
                    at[:],
                    adjt[jb * 512:(jb + 1) * 512, 0:512]
                    .rearrange("(jc2 p) i -> p jc2 i", p=128),
                )
                pre_adj.append(at)
            for gp in range(4):  # own panels of 512 rows -> q
                xq_t = xq_tiles[gp]
                psqs = [ps1.tile([128, D], F32, tag="psk", name=f"psq_{gp}_{i}")
                        for i in range(4)]
                for u in range(2):
                    for t2 in range(4):
                        nc.tensor.matmul(
                            psqs[t2][:],
                            lhsT=xq_t[:, 2 * u:2 * u + 2, t2 * 128:(t2 + 1) * 128],
                            rhs=wq_s[:, 2 * u:2 * u + 2, :],
                            start=(u == 0),
                            stop=(u == 1),
                            perf_mode=DR,
                        )
                for t2 in range(4):
                    if t2 % 2 == 0:
                        nc.vector.tensor_copy(q_s[:, gp * 4 + t2, :], psqs[t2][:])
                    else:
                        nc.scalar.activation(out=q_s[:, gp * 4 + t2, :], in_=psqs[t2][:],
                                             func=mybir.ActivationFunctionType.Copy)

        if stage == "p1":
            with tc.tile_pool(name="dbg", bufs=2) as dbg:
                for t in range(16):
                    dt_ = dbg.tile([128, D], F32, tag="dbg")
                    nc.vector.tensor_copy(dt_[:], km_s[:, t, :])
                    nc.sync.dma_start(loutd[t * 128:(t + 1) * 128, :], dt_[:])
            bigctx.close()

        # ---- Phase 2: agg = adj_block @ k_m ; w = inv_scale * rowdot(q, agg)
        if stage != "p1":
            with (
                tc.tile_pool(name="scrp", bufs=4) as scrpool,
                tc.tile_pool(name="ps2", bufs=8, space="PSUM") as ps2,
            ):
                for qq in range(4):  # quarters of 512 own rows -> 4 PSUM banks
                    aggs = [ps2.tile([128, D], F32, tag="agg", name=f"agg_{qq}_{i}") for i in range(4)]
                    for jb in range(16):  # batches of 4 j-chunks (512 KB DMA)
                        if qq == 0 and jb < 2:
                            at = pre_adj[jb]
                        else:
                            at = s2pool.tile([128, 4, 512], FP8, tag="adjs")
                            nc.sync.dma_start(
                                at[:],
                                adjt[jb * 512:(jb + 1) * 512, qq * 512:(qq + 1) * 512]
                                .rearrange("(jc2 p) i -> p jc2 i", p=128),
                            )
                        for u in range(2):  # chunk pairs -> fp8 DoubleRow (K=256/MM)
                            jp2 = jb * 2 + u
                            for is_ in range(4):
                                nc.tensor.matmul(
                                    aggs[is_][:],
                                    lhsT=at[:, 2 * u:2 * u + 2, is_ * 128:(is_ + 1) * 128],
                                    rhs=km_s[:, 4 * jb + 2 * u:4 * jb + 2 * u + 2, :],
                                    start=(jp2 == 0),
                                    stop=(jp2 == 31),
                                    perf_mode=DR,
                                )
                    for is_ in range(4):
                        t = qq * 4 + is_
                        # NOTE: tensor_tensor_reduce with a PSUM in0 faults the
                        # device (HW-only, sim-clean) — use mul + reduce instead.
                        scr = scrpool.tile([128, D], F32, tag="scr")
                        nc.vector.tensor_mul(scr[:], aggs[is_][:], q_s[:, t, :])
                        nc.vector.reduce_sum(out=w_sb[:, t:t + 1], in_=scr[:], axis=X_AX)
                    # finalize + publish this quarter's w; gathers for quarters
                    # 0-2 hide under the remaining matmul stream
                    qsl = slice(qq * 4, qq * 4 + 4)
                    nc.vector.tensor_scalar_mul(w2[:, qsl], w_sb[:, qsl], INV_SCALE)
                    nc.vector.tensor_mul(w2[:, qsl], w2[:, qsl], maskown_s[:, qsl])
                # own X for the v projection: sync queue AFTER the adj stream
                # so it never steals bandwidth from the phase-1/2 DMAs
                nc.sync.dma_start(xv_s[:], xvt[:].rearrange("(c p) j -> p c j", p=128))
                # single w AllGather at the end: one CC op (latency hides
                # under the v projection), robust to CC slowness
                if stage != "p2":
                    nc.gpsimd.dma_start(w_locd[:], w2[:])
                    nc.gpsimd.collective_compute(
                        "AllGather",
                        mybir.AluOpType.bypass,
                        replica_groups=GROUPS,
                        ins=[w_locd[:]],
                        outs=[w_alld[:]],
                    )

            s2ctx.close()
            bigctx.close()  # frees km/q/wq/wk (128+32+4 KB/partition) for the tail
            if stage == "p2":
                dbg2 = ctx.enter_context(tc.tile_pool(name="dbgp2", bufs=1))
                w2c = dbg2.tile([128, 16], BF16)
                nc.vector.tensor_copy(w2c[:], w2[:])
                nc.sync.dma_start(loutd[0:128, 0:16], w2c[:])

        if stage == "coll":
            with tc.tile_pool(name="dbg2", bufs=1) as dbg2:
                wdbg = dbg2.tile([128, 4, 16], F32)
                wdbgc = dbg2.tile([128, 64], BF16)
                nc.sync.dma_start(wdbg[:], w_alld[:].rearrange("g p t -> p g t"))
                nc.vector.tensor_copy(wdbgc[:], wdbg[:].rearrange("p g t -> p (g t)"))
                nc.sync.dma_start(loutd[0:128, 0:64], wdbgc[:])

        if stage == "full":
            with (
                tc.tile_pool(name="tail", bufs=1) as tailp,
                tc.tile_pool(name="ltp", bufs=8) as ltp,
                tc.tile_pool(name="psv", bufs=4, space="PSUM") as psv,
                tc.tile_pool(name="pst", bufs=1, space="PSUM") as pst,
            ):
                # ---- Phase 3: v = X @ Wv (bf16), covers the last w gather ---
                # chunk pairs interleaved across PSUM banks for MM overlap
                for pr in range(8):
                    pvs = [psv.tile([128, D], F32, tag="psv", name=f"psv_{pr}_{h}")
                           for h in range(2)]
                    for cc in range(4):
                        for h in range(2):
                            t = 2 * pr + h
                            nc.tensor.matmul(
                                pvs[h][:],
                                lhsT=xv_s[:, cc, t * 128:(t + 1) * 128],
                                rhs=wv_s[:, cc, :],
                                start=(cc == 0),
                                stop=(cc == 3),
                            )
                    for h in range(2):
                        nc.vector.tensor_copy(v_sb[:, 2 * pr + h, :], pvs[h][:])

                # ---- Phase 4: softmax normalizers (own batch only) ---------
                wall3 = tailp.tile([128, 4, 16], F32)
                nc.sync.dma_start(wall3[:], w_alld[:].rearrange("g p t -> p g t"))
                wall = wall3[:].rearrange("p g t -> p (g t)")
                # global max via free-dim reduce + PE transpose + reduce
                m1 = tailp.tile([128, 1], F32, tag="m1")
                nc.vector.reduce_max(out=m1[:], in_=wall, axis=X_AX)
                mt_ps = pst.tile([1, 128], F32, tag="mt")
                nc.tensor.transpose(mt_ps[:], m1[:], ident_s[:])
                mt_sb = tailp.tile([1, 128], F32, tag="mts")
                nc.vector.tensor_copy(mt_sb[:], mt_ps[:])
                gmax = tailp.tile([1, 1], F32, tag="gmax")
                nc.vector.reduce_max(out=gmax[:], in_=mt_sb[:], axis=X_AX)
                # broadcast -max to all partitions via (-ones)-matmul
                bc_ps = pst.tile([128, 1], F32, tag="bc")
                nc.tensor.matmul(bc_ps[:], lhsT=negones_r[:], rhs=gmax[:],
                                 start=True, stop=True)
                gneg = tailp.tile([128, 1], F32, tag="gneg")
                nc.vector.tensor_copy(gneg[:], bc_ps[:])
                # sum of exp(w - max) over the bag
                eh = tailp.tile([128, 64], F32, tag="eh")
                nc.scalar.activation(out=eh[:], in_=wall, func=EXP,
                                     bias=gneg[:], scale=1.0)
                # eown in parallel with the eh/S chain (same gneg bias; no
                # Ln here — an EXP->LN switch costs a 1.5us ACT table load)
                eown = tailp.tile([128, 16], F32, tag="eown")
                nc.scalar.activation(out=eown[:], in_=w2[:], func=EXP,
                                     bias=gneg[:], scale=1.0)
                s1 = tailp.tile([128, 1], F32, tag="s1")
                nc.vector.reduce_sum(out=s1[:], in_=eh[:], axis=X_AX)
                ssum_ps = pst.tile([1, 1], F32, tag="ssum")
                nc.tensor.matmul(ssum_ps[:], lhsT=s1[:], rhs=ones_c[:],
                                 start=True, stop=True)
                ssum_sb = tailp.tile([1, 1], F32, tag="ssums")
                nc.vector.tensor_copy(ssum_sb[:], ssum_ps[:])
                rinv1 = tailp.tile([1, 1], F32, tag="rinv1")
                nc.vector.reciprocal(rinv1[:], ssum_sb[:])
                rb_ps = pst.tile([128, 1], F32, tag="rb")
                nc.tensor.matmul(rb_ps[:], lhsT=ones_r[:], rhs=rinv1[:],
                                 start=True, stop=True)
                rinv = tailp.tile([128, 1], F32, tag="rinv")
                nc.vector.tensor_copy(rinv[:], rb_ps[:])
                pown = tailp.tile([128, 16], F32, tag="pown")
                nc.vector.tensor_scalar_mul(pown[:], eown[:], rinv[:, 0:1])

                # ---- Phase 5: L rows = p_i * v_i (bf16 out); muls split over
                # vector + scalar-activation, stores over sync+gpsimd queues -
                for t in range(16):
                    lt = ltp.tile([128, D], BF16, tag="lt")
                    # scalar ACT Copy is ~2.3x slower than vector per op:
                    # give scalar 5 of 16 chunks
                    if t % 3 == 2:
                        nc.scalar.activation(out=lt[:], in_=v_sb[:, t, :],
                                             func=mybir.ActivationFunctionType.Copy,
                                             scale=pown[:, t:t + 1])
                    else:
                        nc.vector.tensor_scalar_mul(lt[:], v_sb[:, t, :], pown[:, t:t + 1])
                    eng = nc.sync if t % 2 == 0 else nc.gpsimd
                    eng.dma_start(loutd[t * 128:(t + 1) * 128, :], lt[:])

    nc.finalize()
    return nc


def _prep_inputs(X, adj, mask, Wqk, Wv):
    import ml_dtypes
    bf16 = ml_dtypes.bfloat16
    fp8 = ml_dtypes.float8_e4m3
    X = np.ascontiguousarray(np.asarray(X, dtype=np.float32))
    adj = np.asarray(adj, dtype=np.float32)
    mask = np.ascontiguousarray(np.asarray(mask, dtype=np.float32))
    Wqk = np.asarray(Wqk, dtype=np.float32)
    Wv = np.ascontiguousarray(np.asarray(Wv, dtype=np.float32))
    wq8_h = np.ascontiguousarray(Wqk[:, :D].astype(fp8))
    wk8_h = np.ascontiguousarray(Wqk[:, D:].astype(fp8))
    wv16_h = np.ascontiguousarray(Wv.astype(bf16))
    ident = np.eye(128, dtype=np.float32)

    in_maps = []
    for b in range(B):
        xt_b8 = np.ascontiguousarray(X[b].T.astype(fp8))
        xt_b16 = np.ascontiguousarray(X[b].T.astype(bf16))
        adjt_bh = np.ascontiguousarray(adj[b].astype(fp8).T)
        maskq_b = np.ascontiguousarray(mask[b].reshape(64, 128).T)
        for r in range(4):
            i0 = r * RPC
            in_maps.append({
                "adjt": np.ascontiguousarray(adjt_bh[:, i0:i0 + RPC]),
                "xt8": xt_b8,
                "xq8": np.ascontiguousarray(xt_b8[:, i0:i0 + RPC]),
                "xvt": np.ascontiguousarray(xt_b16[:, i0:i0 + RPC]),
                "wq8": wq8_h,
                "wk8": wk8_h,
                "wv16": wv16_h,
                "maskq": maskq_b,
                "maskown": np.ascontiguousarray(mask[b, i0:i0 + RPC].reshape(16, 128).T),
                "ident": ident,
            })
    return in_maps


def _run(inputs, stage="full", **kwargs):
    key = f"nc_{stage}"
    if key not in _CACHE:
        _CACHE[key] = _build(stage)
    nc = _CACHE[key]
    in_maps = _prep_inputs(**inputs)
    res = run_bass_kernel_spmd(nc, in_maps, list(range(8)), **kwargs)
    L = np.empty((B, N, D), np.float32)
    for c in range(8):
        b, r = divmod(c, 4)
        L[b, r * RPC:(r + 1) * RPC] = res.results[c]["lout"].astype(np.float32)
    return L, res


def kernel(X, adj, mask, Wqk, Wv):
    L, _ = _run(dict(X=X, adj=adj, mask=mask, Wqk=Wqk, Wv=Wv))
    return L


# revision 39
# speedup vs baseline: 1.0893x; 1.0893x over previous
"""CAMIL self-attention Trainium2 kernel (8 NeuronCores, SPMD).

Reference computation (B=2, N=8192, IN_DIM=ATT_DIM=512):
    q = X @ Wq ; k = X @ Wk ; v = X @ Wv
    w_i = inv_scale * m_i * sum_d q[i,d] * (adj @ (k*m))[i,d]
    L   = softmax(w, axis=bag)[:, :, None] * v

Sharding: 8 cores = (batch 2) x (4 row-blocks of 2048). Each core holds
adj[b, i_block, :]^T (pre-transposed on host so the contraction dim j lands
on SBUF partitions), computes its w-slice on-device, the 4 cores of each
batch AllGather w (8 KB/core, one collective), every core computes the
softmax normalizers locally, then scales its own v rows.

Precision strategy (validated vs the reference in numpy: rel err 8.3e-3 vs
the 2e-2 gate; HW matches the numpy sim exactly since all inputs are
quantized on host): adj, X, Wq, Wk and k_m are all fp8e4m3 so every w-path
matmul runs in DoubleRow mode (K=256/instruction, 2x bf16 throughput); the
softmax over the bag is near one-hot (top-2 gaps ~11-16) so w tolerates
absolute errors of ~3 with small output impact. v = X @ Wv runs in bf16
and L is stored bf16 (host upcasts) since the gate is 2e-2 relative.

Schedule notes (from perfetto traces of prior iterations):
- Matmuls are issued chunk-major so consecutive MMs hit different PSUM
  banks and overlap in the PE pipeline (same-bank accumulation serializes
  at ~477ns/MM vs ~264ns overlapped).
- PSUM drains are split across the vector and scalar engines so neither
  gates PSUM-bank recycling (a lone vector drain at ~654ns/tile makes
  phase 1 vector-bound).
- The v projection runs AFTER the big matmul so its PE work covers the
  single w-AllGather's 15-25us collective latency (one CC op, warmed at
  startup: CC ops are latency-dominated and variable, so fewer is better).
- Softmax partition reductions use PE-transpose + ones-matmuls instead of
  gpsimd partition_all_reduce (avoids a 7us gpsimd library load), and keep
  the scalar engine on EXP only (an EXP->LN switch costs a 1.5us ACT-table
  load).
- The first adj tiles are prefetched before the q projection; bulk const
  loads ride the gpsimd DMA queue; the v-input load rides the sync queue
  behind the adj stream — the xt8/adj streams never stall.
"""

import numpy as np
from contextlib import ExitStack

import concourse.bass as bass
import concourse.bacc as bacc
import concourse.tile as tile
from concourse import mybir, bass_isa
from concourse.bass_utils import run_bass_kernel_spmd

F32 = mybir.dt.float32
BF16 = mybir.dt.bfloat16
FP8 = mybir.dt.float8e4

B, N, D = 2, 8192, 512
RPC = N // 4  # rows per core: 2048
INV_SCALE = float(1.0 / np.sqrt(np.float32(D)))
GROUPS = [[0, 1, 2, 3], [4, 5, 6, 7]]
DR = mybir.MatmulPerfMode.DoubleRow

_CACHE = {}


def _build(stage="full"):
    # stage: debug gate — "p1" (projections only), "p2" (+w), "coll" (+gather),
    # "full" (everything). kernel() always uses "full".
    nc = bacc.Bacc(None, target_bir_lowering=False, debug=False, num_devices=8)

    adjt = nc.dram_tensor("adjt", [N, RPC], FP8, kind="ExternalInput")
    xt8 = nc.dram_tensor("xt8", [D, N], FP8, kind="ExternalInput")
    xq8 = nc.dram_tensor("xq8", [D, RPC], FP8, kind="ExternalInput")
    xvt = nc.dram_tensor("xvt", [D, RPC], BF16, kind="ExternalInput")
    wq8d = nc.dram_tensor("wq8", [D, D], FP8, kind="ExternalInput")
    wk8d = nc.dram_tensor("wk8", [D, D], FP8, kind="ExternalInput")
    wv16d = nc.dram_tensor("wv16", [D, D], BF16, kind="ExternalInput")
    maskqd = nc.dram_tensor("maskq", [128, 64], F32, kind="ExternalInput")
    maskownd = nc.dram_tensor("maskown", [128, 16], F32, kind="ExternalInput")
    identd = nc.dram_tensor("ident", [128, 128], F32, kind="ExternalInput")
    loutd = nc.dram_tensor("lout", [RPC, D], BF16, kind="ExternalOutput")

    w_locd = nc.dram_tensor("w_loc", [128, 16], F32)
    w_alld = nc.dram_tensor("w_all", [4, 128, 16], F32)
    dwarm_in = nc.dram_tensor("dwarm_in", [1, 16], F32)
    dwarm_out = nc.dram_tensor("dwarm_out", [4, 1, 16], F32)

    MUL = mybir.AluOpType.mult
    X_AX = mybir.AxisListType.X
    EXP = mybir.ActivationFunctionType.Exp

    with tile.TileContext(nc) as tc, ExitStack() as ctx:
        wtail = ctx.enter_context(tc.tile_pool(name="wtail", bufs=1))
        bigctx = ExitStack()
        big = bigctx.enter_context(tc.tile_pool(name="big", bufs=1))

        km_s = big.tile([128, 64, D], FP8)       # k*mask, [j-part, j-chunk, d]
        q_s = big.tile([128, 16, D], F32)        # own q rows
        wq_s = big.tile([128, 4, D], FP8)
        wk_s = big.tile([128, 4, D], FP8)
        maskq_s = big.tile([128, 64], F32)

        w_sb = wtail.tile([128, 16], F32)        # own w (pre-mask)
        w2 = wtail.tile([128, 16], F32)          # own w (masked)
        maskown_s = wtail.tile([128, 16], F32)
        ident_s = wtail.tile([128, 128], F32)
        ones_c = wtail.tile([128, 1], F32)       # column of ones (dot lhsT)
        ones_r = wtail.tile([1, 128], F32)       # row of ones (broadcast lhsT)
        negones_r = wtail.tile([1, 128], F32)    # row of -1 (negating broadcast)
        wv_s = wtail.tile([128, 4, D], BF16)
        xv_s = wtail.tile([128, 4, RPC], BF16)   # own X^T bf16 for v
        v_sb = wtail.tile([128, 16, D], BF16)    # own v rows

        def _late_consts():
            # deferred constant loads on the gpsimd DMA queue so they never
            # stall the xt8 stream; plus exp-LUT warmup on idle engines
            nc.gpsimd.dma_start(wq_s[:], wq8d[:].rearrange("(c p) d -> p c d", p=128))
            nc.gpsimd.dma_start(wv_s[:], wv16d[:].rearrange("(c p) d -> p c d", p=128))
            nc.gpsimd.dma_start(maskown_s[:], maskownd[:])
            nc.gpsimd.dma_start(ident_s[:], identd[:])
            nc.vector.memset(ones_c[:], 1.0)
            nc.vector.memset(ones_r[:], 1.0)
            nc.vector.memset(negones_r[:], -1.0)
            warm = wtail.tile([128, 16], F32, name="warm")
            nc.vector.memset(warm[:], 0.0)
            nc.scalar.activation(out=warm[:], in_=warm[:], func=EXP,
                                 bias=0.0, scale=1.0)
            # collective firmware warmup so the real w-gather is warm
            nc.gpsimd.dma_start(dwarm_in[:], warm[0:1, :])
            nc.gpsimd.collective_compute(
                "AllGather",
                mybir.AluOpType.bypass,
                replica_groups=GROUPS,
                ins=[dwarm_in[:]],
                outs=[dwarm_out[:]],
            )

        # ---- Phase 1: k_m (all N rows) and q (own rows), fp8 DoubleRow ----
        s2ctx = ExitStack()
        s2pool = s2ctx.enter_context(tc.tile_pool(name="s2", bufs=24))
        with (
            tc.tile_pool(name="p1", bufs=8) as p1pool,
            tc.tile_pool(name="ps1", bufs=8, space="PSUM") as ps1,
        ):
            # wk/maskq ride the idle scalar/gpsimd queues so the sync queue's
            # first (cold, ~1.5us) DMAs are the xt8 panel halves; xq8 stays on
            # sync — on scalar it queues behind the k-loop's ACT drains, on a
            # cold queue it steals panel-stream bandwidth (both measured)
            nc.scalar.dma_start(wk_s[:], wk8d[:].rearrange("(c p) d -> p c d", p=128))
            nc.gpsimd.dma_start(maskq_s[:], maskqd[:])
            for jp in range(16):  # panels of 512 bag rows (split DMA halves
                # so the first DR matmul starts after 128KB, not 256KB)
                xt_t = p1pool.tile([128, 4, 512], FP8, tag="xtp")
                src = xt8[:, jp * 512:(jp + 1) * 512].rearrange("(c p) j -> p c j", p=128)
                nc.sync.dma_start(xt_t[:, 0:2, :], src[:, 0:2, :])
                nc.sync.dma_start(xt_t[:, 2:4, :], src[:, 2:4, :])
                if jp == 1:
                    _late_consts()
                # chunk-major matmul order so consecutive MMs hit different
                # PSUM banks and overlap in the PE pipeline
                psks = [ps1.tile([128, D], F32, tag="psk", name=f"psk_{jp}_{i}")
                        for i in range(4)]
                for u in range(2):
                    for jc2 in range(4):
                        nc.tensor.matmul(
                            psks[jc2][:],
                            lhsT=xt_t[:, 2 * u:2 * u + 2, jc2 * 128:(jc2 + 1) * 128],
                            rhs=wk_s[:, 2 * u:2 * u + 2, :],
                            start=(u == 0),
                            stop=(u == 1),
                            perf_mode=DR,
                        )
                for jc2 in range(4):
                    jc = jp * 4 + jc2
                    # PSUM drains split across vector + scalar so neither
                    # engine gates the PE's PSUM-bank recycling
                    if jc2 % 2 == 0:
                        nc.vector.tensor_scalar_mul(km_s[:, jc, :], psks[jc2][:], maskq_s[:, jc:jc + 1])
                    else:
                        nc.scalar.activation(out=km_s[:, jc, :], in_=psks[jc2][:],
                                             func=mybir.ActivationFunctionType.Copy,
                                             scale=maskq_s[:, jc:jc + 1])
            # prefetch the first adj tiles so phase 2 starts right after q
            pre_adj = []
            for jb in range(2):
                at = s2pool.tile([128, 4, 512], FP8, tag="adjs", name=f"adjpre{jb}")
                nc.sync.dma_start(
                    at[:],
                    adjt[jb * 512:(jb + 1) * 512, 0:512]
                    .rearrange("(jc2 p) i -> p jc2 i", p=128),
                )
                pre_adj.append(at)
            for gp in range(4):  # own panels of 512 rows -> q
                xq_t = p1pool.tile([128, 4, 512], FP8, tag="xtp")
                nc.sync.dma_start(
                    xq_t[:],
                    xq8[:, gp * 512:(gp + 1) * 512].rearrange("(c p) j -> p c j", p=128),
                )
                psqs = [ps1.tile([128, D], F32, tag="psk", name=f"psq_{gp}_{i}")
                        for i in range(4)]
                for u in range(2):
                    for t2 in range(4):
                        nc.tensor.matmul(
                            psqs[t2][:],
                            lhsT=xq_t[:, 2 * u:2 * u + 2, t2 * 128:(t2 + 1) * 128],
                            rhs=wq_s[:, 2 * u:2 * u + 2, :],
                            start=(u == 0),
                            stop=(u == 1),
                            perf_mode=DR,
                        )
                for t2 in range(4):
                    if t2 % 2 == 0:
                        nc.vector.tensor_copy(q_s[:, gp * 4 + t2, :], psqs[t2][:])
                    else:
                        nc.scalar.activation(out=q_s[:, gp * 4 + t2, :], in_=psqs[t2][:],
                                             func=mybir.ActivationFunctionType.Copy)

        if stage == "p1":
            with tc.tile_pool(name="dbg", bufs=2) as dbg:
                for t in range(16):
                    dt_ = dbg.tile([128, D], F32, tag="dbg")
                    nc.vector.tensor_copy(dt_[:], km_s[:, t, :])
                    nc.sync.dma_start(loutd[t * 128:(t + 1) * 128, :], dt_[:])
            bigctx.close()

        # ---- Phase 2: agg = adj_block @ k_m ; w = inv_scale * rowdot(q, agg)
        if stage != "p1":
            with (
                tc.tile_pool(name="scrp", bufs=4) as scrpool,
                tc.tile_pool(name="ps2", bufs=8, space="PSUM") as ps2,
            ):
                for qq in range(4):  # quarters of 512 own rows -> 4 PSUM banks
                    aggs = [ps2.tile([128, D], F32, tag="agg", name=f"agg_{qq}_{i}") for i in range(4)]
                    for jb in range(16):  # batches of 4 j-chunks (512 KB DMA)
                        if qq == 0 and jb < 2:
                            at = pre_adj[jb]
                        else:
                            at = s2pool.tile([128, 4, 512], FP8, tag="adjs")
                            nc.sync.dma_start(
                                at[:],
                                adjt[jb * 512:(jb + 1) * 512, qq * 512:(qq + 1) * 512]
                                .rearrange("(jc2 p) i -> p jc2 i", p=128),
                            )
                        for u in range(2):  # chunk pairs -> fp8 DoubleRow (K=256/MM)
                            jp2 = jb * 2 + u
                            for is_ in range(4):
                                nc.tensor.matmul(
                                    aggs[is_][:],
                                    lhsT=at[:, 2 * u:2 * u + 2, is_ * 128:(is_ + 1) * 128],
                                    rhs=km_s[:, 4 * jb + 2 * u:4 * jb + 2 * u + 2, :],
                                    start=(jp2 == 0),
                                    stop=(jp2 == 31),
                                    perf_mode=DR,
                                )
                    for is_ in range(4):
                        t = qq * 4 + is_
                        # NOTE: tensor_tensor_reduce with a PSUM in0 faults the
                        # device (HW-only, sim-clean) — use mul + reduce instead.
                        scr = scrpool.tile([128, D], F32, tag="scr")
                        nc.vector.tensor_mul(scr[:], aggs[is_][:], q_s[:, t, :])
                        nc.vector.reduce_sum(out=w_sb[:, t:t + 1], in_=scr[:], axis=X_AX)
                    # finalize + publish this quarter's w; gathers for quarters
                    # 0-2 hide under the remaining matmul stream
                    qsl = slice(qq * 4, qq * 4 + 4)
                    nc.vector.tensor_scalar_mul(w2[:, qsl], w_sb[:, qsl], INV_SCALE)
                    nc.vector.tensor_mul(w2[:, qsl], w2[:, qsl], maskown_s[:, qsl])
                # own X for the v projection: sync queue AFTER the adj stream
                # so it never steals bandwidth from the phase-1/2 DMAs
                nc.sync.dma_start(xv_s[:], xvt[:].rearrange("(c p) j -> p c j", p=128))
                # single w AllGather at the end: one CC op (latency hides
                # under the v projection), robust to CC slowness
                if stage != "p2":
                    nc.gpsimd.dma_start(w_locd[:], w2[:])
                    nc.gpsimd.collective_compute(
                        "AllGather",
                        mybir.AluOpType.bypass,
                        replica_groups=GROUPS,
                        ins=[w_locd[:]],
                        outs=[w_alld[:]],
                    )

            s2ctx.close()
            bigctx.close()  # frees km/q/wq/wk (128+32+4 KB/partition) for the tail
            if stage == "p2":
                dbg2 = ctx.enter_context(tc.tile_pool(name="dbgp2", bufs=1))
                w2c = dbg2.tile([128, 16], BF16)
                nc.vector.tensor_copy(w2c[:], w2[:])
                nc.sync.dma_start(loutd[0:128, 0:16], w2c[:])

        if stage == "coll":
            with tc.tile_pool(name="dbg2", bufs=1) as dbg2:
                wdbg = dbg2.tile([128, 4, 16], F32)
                wdbgc = dbg2.tile([128, 64], BF16)
                nc.sync.dma_start(wdbg[:], w_alld[:].rearrange("g p t -> p g t"))
                nc.vector.tensor_copy(wdbgc[:], wdbg[:].rearrange("p g t -> p (g t)"))
                nc.sync.dma_start(loutd[0:128, 0:64], wdbgc[:])

        if stage == "full":
            with (
                tc.tile_pool(name="tail", bufs=1) as tailp,
                tc.tile_pool(name="ltp", bufs=8) as ltp,
                tc.tile_pool(name="psv", bufs=4, space="PSUM") as psv,
                tc.tile_pool(name="pst", bufs=1, space="PSUM") as pst,
            ):
                # ---- Phase 3: v = X @ Wv (bf16), covers the last w gather ---
                # chunk pairs interleaved across PSUM banks for MM overlap
                for pr in range(8):
                    pvs = [psv.tile([128, D], F32, tag="psv", name=f"psv_{pr}_{h}")
                           for h in range(2)]
                    for cc in range(4):
                        for h in range(2):
                            t = 2 * pr + h
                            nc.tensor.matmul(
                                pvs[h][:],
                                lhsT=xv_s[:, cc, t * 128:(t + 1) * 128],
                                rhs=wv_s[:, cc, :],
                                start=(cc == 0),
                                stop=(cc == 3),
                            )
                    for h in range(2):
                        nc.vector.tensor_copy(v_sb[:, 2 * pr + h, :], pvs[h][:])

                # ---- Phase 4: softmax normalizers (own batch only) ---------
                wall3 = tailp.tile([128, 4, 16], F32)
                nc.sync.dma_start(wall3[:], w_alld[:].rearrange("g p t -> p g t"))
                wall = wall3[:].rearrange("p g t -> p (g t)")
                # global max via free-dim reduce + PE transpose + reduce
                m1 = tailp.tile([128, 1], F32, tag="m1")
                nc.vector.reduce_max(out=m1[:], in_=wall, axis=X_AX)
                mt_ps = pst.tile([1, 128], F32, tag="mt")
                nc.tensor.transpose(mt_ps[:], m1[:], ident_s[:])
                mt_sb = tailp.tile([1, 128], F32, tag="mts")
                nc.vector.tensor_copy(mt_sb[:], mt_ps[:])
                gmax = tailp.tile([1, 1], F32, tag="gmax")
                nc.vector.reduce_max(out=gmax[:], in_=mt_sb[:], axis=X_AX)
                # broadcast -max to all partitions via (-ones)-matmul
                bc_ps = pst.tile([128, 1], F32, tag="bc")
                nc.tensor.matmul(bc_ps[:], lhsT=negones_r[:], rhs=gmax[:],
                                 start=True, stop=True)
                gneg = tailp.tile([128, 1], F32, tag="gneg")
                nc.vector.tensor_copy(gneg[:], bc_ps[:])
                # sum of exp(w - max) over the bag
                eh = tailp.tile([128, 64], F32, tag="eh")
                nc.scalar.activation(out=eh[:], in_=wall, func=EXP,
                                     bias=gneg[:], scale=1.0)
                # eown in parallel with the eh/S chain (same gneg bias; no
                # Ln here — an EXP->LN switch costs a 1.5us ACT table load)
                eown = tailp.tile([128, 16], F32, tag="eown")
                nc.scalar.activation(out=eown[:], in_=w2[:], func=EXP,
                                     bias=gneg[:], scale=1.0)
                s1 = tailp.tile([128, 1], F32, tag="s1")
                nc.vector.reduce_sum(out=s1[:], in_=eh[:], axis=X_AX)
                ssum_ps = pst.tile([1, 1], F32, tag="ssum")
                nc.tensor.matmul(ssum_ps[:], lhsT=s1[:], rhs=ones_c[:],
                                 start=True, stop=True)
                ssum_sb = tailp.tile([1, 1], F32, tag="ssums")
                nc.vector.tensor_copy(ssum_sb[:], ssum_ps[:])
                rinv1 = tailp.tile([1, 1], F32, tag="rinv1")
                nc.vector.reciprocal(rinv1[:], ssum_sb[:])
                rb_ps = pst.tile([128, 1], F32, tag="rb")
                nc.tensor.matmul(rb_ps[:], lhsT=ones_r[:], rhs=rinv1[:],
                                 start=True, stop=True)
                rinv = tailp.tile([128, 1], F32, tag="rinv")
                nc.vector.tensor_copy(rinv[:], rb_ps[:])
                pown = tailp.tile([128, 16], F32, tag="pown")
                nc.vector.tensor_scalar_mul(pown[:], eown[:], rinv[:, 0:1])

                # ---- Phase 5: L rows = p_i * v_i (bf16 out); muls split over
                # vector + scalar-activation, stores over sync+gpsimd queues -
                for t in range(16):
                    lt = ltp.tile([128, D], BF16, tag="lt")
                    # scalar ACT Copy is ~2.3x slower than vector per op:
                    # give scalar 5 of 16 chunks
                    if t % 3 == 2:
                        nc.scalar.activation(out=lt[:], in_=v_sb[:, t, :],
                                             func=mybir.ActivationFunctionType.Copy,
                                             scale=pown[:, t:t + 1])
                    else:
                        nc.vector.tensor_scalar_mul(lt[:], v_sb[:, t, :], pown[:, t:t + 1])
                    eng = nc.sync if t % 2 == 0 else nc.gpsimd
                    eng.dma_start(loutd[t * 128:(t + 1) * 128, :], lt[:])

    nc.finalize()
    return nc


def _prep_inputs(X, adj, mask, Wqk, Wv):
    import ml_dtypes
    bf16 = ml_dtypes.bfloat16
    fp8 = ml_dtypes.float8_e4m3
    X = np.ascontiguousarray(np.asarray(X, dtype=np.float32))
    adj = np.asarray(adj, dtype=np.float32)
    mask = np.ascontiguousarray(np.asarray(mask, dtype=np.float32))
    Wqk = np.asarray(Wqk, dtype=np.float32)
    Wv = np.ascontiguousarray(np.asarray(Wv, dtype=np.float32))
    wq8_h = np.ascontiguousarray(Wqk[:, :D].astype(fp8))
    wk8_h = np.ascontiguousarray(Wqk[:, D:].astype(fp8))
    wv16_h = np.ascontiguousarray(Wv.astype(bf16))
    ident = np.eye(128, dtype=np.float32)

    in_maps = []
    for b in range(B):
        xt_b8 = np.ascontiguousarray(X[b].T.astype(fp8))
        xt_b16 = np.ascontiguousarray(X[b].T.astype(bf16))
        adjt_bh = np.ascontiguousarray(adj[b].astype(fp8).T)
        maskq_b = np.ascontiguousarray(mask[b].reshape(64, 128).T)
        for r in range(4):
            i0 = r * RPC
            in_maps.append({
                "adjt": np.ascontiguousarray(adjt_bh[:, i0:i0 + RPC]),
                "xt8": xt_b8,
                "xq8": np.ascontiguousarray(xt_b8[:, i0:i0 + RPC]),
                "xvt": np.ascontiguousarray(xt_b16[:, i0:i0 + RPC]),
                "wq8": wq8_h,
                "wk8": wk8_h,
                "wv16": wv16_h,
                "maskq": maskq_b,
                "maskown": np.ascontiguousarray(mask[b, i0:i0 + RPC].reshape(16, 128).T),
                "ident": ident,
            })
    return in_maps


def _run(inputs, stage="full", **kwargs):
    key = f"nc_{stage}"
    if key not in _CACHE:
        _CACHE[key] = _build(stage)
    nc = _CACHE[key]
    in_maps = _prep_inputs(**inputs)
    res = run_bass_kernel_spmd(nc, in_maps, list(range(8)), **kwargs)
    L = np.empty((B, N, D), np.float32)
    for c in range(8):
        b, r = divmod(c, 4)
        L[b, r * RPC:(r + 1) * RPC] = res.results[c]["lout"].astype(np.float32)
    return L, res


def kernel(X, adj, mask, Wqk, Wv):
    L, _ = _run(dict(X=X, adj=adj, mask=mask, Wqk=Wqk, Wv=Wv))
    return L
